# revision 46
# baseline (speedup 1.0000x reference)
# Distributed causal self-attention (RoPE) kernel for one TRN2 chip (8 NeuronCores).
#
# Sharding: data-parallel over batch (B=2) x tensor-parallel over heads
# (16 heads -> 4 per core).  Core i handles batch i//4, heads (i%4)*4 .. +4.
# Each core computes its 4 heads' attention and a partial output projection
# (row-parallel Wo); partials are summed on the host (the only "collective").
#
# Device algorithm (per core, all matmuls bf16 with f32 PSUM accumulation):
#   qkT = wqkT.T @ xT            [512, S]  (q,k in head-transposed layout [d, s])
#   V   = xT.T @ wvT             [S, 256]  (natural layout, +ones column per head)
#   RoPE on qkT (swap via SBUF->SBUF DMA, mults on DVE)
#   per head pair, per 512-wide q quarter, per k-tile (128):
#       scoresT[k, q] = kT_tile.T @ qT      (two heads packed in PE row groups)
#       probsT = exp(scoresT/8)             (ScalarE, no max-subtraction: |s|<=9)
#       triangular 0/1 mask on the diagonal block (DVE)
#       outT[d, q] += Vaug[k-tile].T @ probsT   (65th row accumulates softmax sums)
#   normalize: nao = outT[0:64] * broadcast(1/outT[64])
#   partial_out = naoT.T @ woT   [S, 1024] f32 -> DMA out
import numpy as np
import ml_dtypes

S = 2048
H = 1024
HD = 64
NCORES = 8
BF = ml_dtypes.bfloat16

_CACHE = {}


def _rope_tables():
    inv = 1.0 / (10000.0 ** (np.arange(0, HD, 2, dtype=np.float64) / HD))
    fr = np.outer(np.arange(S, dtype=np.float64), inv)  # [S, 32]
    cosT = np.cos(fr).T.astype(np.float32)  # [32, S]
    sinT = np.sin(fr).T.astype(np.float32)
    crep = np.concatenate([cosT] * 4, 0)  # [128, S]
    srep = np.concatenate([-sinT, sinT, -sinT, sinT], 0)
    return crep.astype(BF), srep.astype(BF)


def _build_program(debug_outputs=False):
    import concourse.bass as bass
    import concourse.mybir as mybir
    import concourse.tile as tile
    from concourse import bacc

    DT = mybir.dt.bfloat16
    F32 = mybir.dt.float32
    EXP = mybir.ActivationFunctionType.Exp

    nc = bacc.Bacc(
        "TRN2",
        target_bir_lowering=False,
        debug=False,
        enable_asserts=False,
        num_devices=NCORES,
    )
    if debug_outputs:
        dbg_qkr_d = nc.declare_dram_parameter("dbg_qkr", [4, 128, S], DT, isOutput=True)
        dbg_v_d = nc.declare_dram_parameter("dbg_v", [128, 16 * 4 * 65], DT, isOutput=True)
        dbg_nao_d = nc.declare_dram_parameter("dbg_nao", [128, 2, S], DT, isOutput=True)
        dbg_rcp_d = nc.declare_dram_parameter("dbg_rcp", [1, 512], F32, isOutput=True)
        dbg_bc_d = nc.declare_dram_parameter("dbg_bc", [64, 512], F32, isOutput=True)
        dbg_ops_d = nc.declare_dram_parameter("dbg_ops", [65, 512], F32, isOutput=True)
        dbg_pr_d = nc.declare_dram_parameter("dbg_pr", [4, 128, 2, 512], DT, isOutput=True)
    xT_d = nc.declare_dram_parameter("xT", [H, S], DT, isOutput=False)
    wqkT_d = nc.declare_dram_parameter("wqkT", [H, 512], DT, isOutput=False)
    wvT_d = nc.declare_dram_parameter("wvT", [H, 256], DT, isOutput=False)
    woT_d = nc.declare_dram_parameter("woT", [256, H], DT, isOutput=False)
    crep_d = nc.declare_dram_parameter("crep", [128, S], DT, isOutput=False)
    srep_d = nc.declare_dram_parameter("srep", [128, S], DT, isOutput=False)
    tri_d = nc.declare_dram_parameter("trimask", [128, 128], DT, isOutput=False)
    out_d = nc.declare_dram_parameter("out", [S, H], F32, isOutput=True)

    with tile.TileContext(nc) as tc:
        with (
            tc.tile_pool(name="consts", bufs=1) as consts,
            tc.tile_pool(name="rtmp", bufs=1) as rtmp,
            tc.tile_pool(name="qkpool", bufs=2) as qkpool,
            tc.tile_pool(name="probs", bufs=6) as prpool,
            tc.tile_pool(name="small", bufs=4) as small,
            tc.tile_pool(name="fout", bufs=3) as foutp,
            tc.tile_pool(name="qkvps", bufs=2, space="PSUM") as qkvps,
            tc.tile_pool(name="spps", bufs=2, space="PSUM") as spps,
            tc.tile_pool(name="opps", bufs=2, space="PSUM") as opps,
        ):
            # ---- load constants / inputs ----
            # weights first (small), then x streamed per contraction tile so
            # the first QKV matmuls start within a few us
            x_ap = xT_d.ap().rearrange("(k c p) s -> k p c s", p=128, c=2)
            wqk_all = consts.tile([128, 8, 512], DT, name="wqk_all")
            nc.sync.dma_start(wqk_all, wqkT_d.ap().rearrange("(c p) o -> p c o", p=128))
            wqk_sb = [wqk_all[:, ct, :] for ct in range(8)]
            x_sb = []
            for k in range(4):
                xt = consts.tile([128, 2, S], DT, name=f"x_sb{k}")
                nc.sync.dma_start(xt, x_ap[k])
                x_sb.append(xt[:, 0, :])
                x_sb.append(xt[:, 1, :])
            wv_sb = consts.tile([128, 8, 256], DT, name="wv_sb")
            nc.sync.dma_start(wv_sb, wvT_d.ap().rearrange("(c p) o -> p c o", p=128))
            wo_sb = consts.tile([128, 2, H], DT, name="wo_sb")
            nc.sync.dma_start(wo_sb, woT_d.ap().rearrange("(c p) o -> p c o", p=128))
            crep_sb = consts.tile([128, S], DT, name="crep_sb")
            nc.sync.dma_start(crep_sb, crep_d.ap())
            srep_sb = consts.tile([128, S], DT, name="srep_sb")
            nc.sync.dma_start(srep_sb, srep_d.ap())
            tri_sb = consts.tile([128, 2, 128], DT, name="tri_sb")
            nc.sync.dma_start(tri_sb[:, 0, :], tri_d.ap())
            nc.sync.dma_start(tri_sb[:, 1, :], tri_d.ap())

            v_sb = consts.tile([128, 16, 4, 65], DT, name="v_sb")
            nc.gpsimd.memset(v_sb[:, :, :, 64:65], 1.0)
            ones_sb = consts.tile([1, 64], DT, name="ones_sb")
            nc.gpsimd.memset(ones_sb, 1.0)
            warm_sb = consts.tile([128, 512], DT, name="warm_sb")
            nc.gpsimd.memset(warm_sb, 0.0)

            def warm_pe(n, rhs=None):
                # scratch matmuls keeping the PE HAM activity window busy while
                # it would otherwise idle (DMA-bound startup, end-of-kernel
                # normalization chain) so real matmuls run at 2.4 GHz instead
                # of the cold 1.2 GHz.  An rhs written by earlier work anchors
                # the scheduler so these land in the intended window.
                wps = qkvps.tile([128, 512], F32, tag="qkvps", name=f"warm{warm_pe.i}")
                warm_pe.i += 1
                if rhs is None:
                    rhs = warm_sb
                for _ in range(n):
                    nc.tensor.matmul(wps, warm_sb[:, 0:128], rhs, start=True, stop=True)
            warm_pe.i = 0

            qk = {}   # raw qkT blocks: 0=q01 1=q23 2=k01 3=k23
            qkw = {}  # half-swapped
            qkr = {}  # roped
            for ot in range(4):
                qk[ot] = consts.tile([128, S], DT, name=f"qk{ot}")
                qkw[ot] = consts.tile([128, S], DT, name=f"qkw{ot}")
                qkr[ot] = consts.tile([128, S], DT, name=f"qkr{ot}")
            # normalized attention output, keyed (ctile, quarter); ctile c
            # holds heads 2c,2c+1 stacked on partitions
            nao = {
                (ct2, c): consts.tile([128, 512], DT, name=f"nao{ct2}{c}")
                for ct2 in range(2)
                for c in range(4)
            }

            def qkv_chunk(ot, scn):
                sl = slice(scn * 512, (scn + 1) * 512)
                ps = qkvps.tile([128, 512], F32, tag="qkvps")
                for ct in range(8):
                    nc.tensor.matmul(
                        ps,
                        wqk_sb[ct][:, ot * 128:(ot + 1) * 128],
                        x_sb[ct][:, sl],
                        start=(ct == 0),
                        stop=(ct == 7),
                    )
                nc.vector.tensor_copy(qk[ot][:, sl], ps)
                # swap head-dim halves (partition permute) via SBUF->SBUF DMA
                for hh in range(2):
                    b0 = hh * 64
                    nc.sync.dma_start(qkw[ot][b0:b0 + 32, sl], qk[ot][b0 + 32:b0 + 64, sl])
                    nc.sync.dma_start(qkw[ot][b0 + 32:b0 + 64, sl], qk[ot][b0:b0 + 32, sl])
                t1 = rtmp.tile([128, 512], DT, tag="rt1", bufs=2)
                t2 = rtmp.tile([128, 512], DT, tag="rt2", bufs=2)
                nc.vector.tensor_mul(t1, qk[ot][:, sl], crep_sb[:, sl])
                nc.vector.tensor_mul(t2, qkw[ot][:, sl], srep_sb[:, sl])
                nc.vector.tensor_add(qkr[ot][:, sl], t1, t2)

            def v_group(st):
                ps = qkvps.tile([128, 256], F32, tag="qkvps")
                for ct in range(8):
                    nc.tensor.matmul(
                        ps,
                        x_sb[ct][:, st * 128:(st + 1) * 128],
                        wv_sb[:, ct, :],
                        start=(ct == 0),
                        stop=(ct == 7),
                    )
                nc.vector.tensor_copy(
                    v_sb[:, st, :, 0:64], ps.rearrange("p (h d) -> p h d", h=4)
                )

            last_pr = [None]

            def attention_quarter(p, c, filler=None):
                # heads hA = 2p, hB = 2p+1 live at partitions 0:64 / 64:128 of
                # qkr[p] (q) and qkr[2+p] (k)
                ops = []
                for hh in range(2):
                    ops.append(
                        opps.tile([65, 512], F32, tag="opps", name=f"op{p}{c}{hh}")
                    )
                nkt = 4 * c + 4
                for kt in range(nkt):
                    if filler is not None and kt in filler:
                        filler[kt]()
                    off = max(0, kt * 128 - c * 512)
                    sp = spps.tile([128, 2, 512], F32, tag="spps")
                    for hh in range(2):
                        nc.tensor.matmul(
                            sp[:, hh, off:512],
                            qkr[2 + p][hh * 64:(hh + 1) * 64, kt * 128:(kt + 1) * 128],
                            qkr[p][hh * 64:(hh + 1) * 64, c * 512 + off:(c + 1) * 512],
                            start=True,
                            stop=True,
                        )
                    pr = prpool.tile([128, 2, 512], DT, tag="probs")
                    last_pr[0] = pr
                    nc.scalar.activation(
                        pr[:, :, off:512], sp[:, :, off:512], EXP, scale=0.125
                    )
                    if kt >= 4 * c:  # diagonal block lives in this quarter
                        dof = (kt - 4 * c) * 128
                        nc.vector.tensor_mul(
                            pr[:, :, dof:dof + 128],
                            pr[:, :, dof:dof + 128],
                            tri_sb,
                        )
                    for hh in range(2):
                        nc.tensor.matmul(
                            ops[hh][:, off:512],
                            v_sb[:, kt, 2 * p + hh, :],
                            pr[:, hh, off:512],
                            start=(kt == 0),
                            stop=(kt == nkt - 1),
                        )
                # single pass over each ops psum tile releases it for the
                # next quarter; the normalization chain then runs off-PSUM
                ao2 = small.tile([64, 2, 512], F32, tag="ao", name=f"ao{p}{c}", bufs=3)
                srow = small.tile([1, 2, 512], F32, tag="srow", bufs=3)
                for hh in range(2):
                    nc.vector.tensor_copy(ao2[:, hh, :], ops[hh][0:64, :])
                    nc.scalar.copy(srow[:, hh, :], ops[hh][64:65, :])
                rcp = small.tile([1, 2, 512], F32, tag="rcp", bufs=3)
                nc.vector.reciprocal_approx_fast(rcp, srow)
                bc = small.tile([64, 2, 512], F32, tag="bc", bufs=3)
                nc.gpsimd.partition_broadcast(bc, rcp)
                for hh in range(2):
                    nc.vector.tensor_mul(
                        nao[(p, c)][hh * 64:(hh + 1) * 64, :],
                        ao2[:, hh, :],
                        bc[:, hh, :],
                    )

            out_ap = out_d.ap()

            def wo_group(c):
                for st in range(4 * c, 4 * c + 4):
                    fo = foutp.tile([128, H], F32, tag="fout")
                    for oh in range(2):
                        fp = qkvps.tile([128, 512], F32, tag="qkvps")
                        for ct2 in range(2):
                            nc.tensor.matmul(
                                fp,
                                nao[(ct2, c)][:, (st % 4) * 128:(st % 4 + 1) * 128],
                                wo_sb[:, ct2, oh * 512:(oh + 1) * 512],
                                start=(ct2 == 0),
                                stop=(ct2 == 1),
                            )
                        if oh == 0:
                            nc.vector.tensor_copy(fo[:, oh * 512:(oh + 1) * 512], fp)
                        else:
                            nc.scalar.copy(fo[:, oh * 512:(oh + 1) * 512], fp)
                    nc.sync.dma_start(out_ap[st * 128:(st + 1) * 128, :], fo)

            # ---- program order (scheduling priority) ----
            # fine-grained interleave: attention quarters start as soon as the
            # q/k columns and V rows they need exist; QKV/V/Wo chunks are the
            # PE filler during the exp-paced attention stream.  The two head
            # pairs alternate quarters so each boundary chain hides inside the
            # other pair's quarter; late wo groups are injected inside the
            # exp-heavy q3 loops.
            warm_pe(28)
            qkv_chunk(0, 0); qkv_chunk(2, 0)
            for st in range(4):
                v_group(st)
            attention_quarter(0, 0)
            qkv_chunk(1, 0); qkv_chunk(3, 0)
            qkv_chunk(0, 1); qkv_chunk(2, 1)
            for st in range(4, 8):
                v_group(st)
            attention_quarter(1, 0)
            qkv_chunk(1, 1); qkv_chunk(3, 1)
            qkv_chunk(0, 2); qkv_chunk(2, 2)
            for st in range(8, 12):
                v_group(st)
            attention_quarter(0, 1)
            qkv_chunk(1, 2); qkv_chunk(3, 2)
            qkv_chunk(0, 3); qkv_chunk(2, 3)
            for st in range(12, 16):
                v_group(st)
            attention_quarter(1, 1)
            qkv_chunk(1, 3); qkv_chunk(3, 3)
            attention_quarter(0, 2)
            wo_group(0)
            attention_quarter(1, 2)
            attention_quarter(0, 3, filler={6: lambda: wo_group(1)})
            attention_quarter(1, 3, filler={6: lambda: wo_group(2)})
            warm_pe(29, rhs=last_pr[0][:, 0, :])
            wo_group(3)

            if debug_outputs:
                for ot in range(4):
                    nc.sync.dma_start(dbg_qkr_d.ap()[ot], qkr[ot])
                nc.sync.dma_start(
                    dbg_v_d.ap(), v_sb.rearrange("p a b c -> p (a b c)")
                )
                for ct2 in range(2):
                    for c in range(4):
                        nc.sync.dma_start(
                            dbg_nao_d.ap()[:, ct2, c * 512:(c + 1) * 512],
                            nao[(ct2, c)],
                        )

    nc.compile()
    return nc


def _get_program(debug_outputs=False):
    key = ("nc", debug_outputs)
    if key not in _CACHE:
        _CACHE[key] = _build_program(debug_outputs)
    return _CACHE[key]


def make_in_maps(hidden_states, Wqkv, Wo):
    hs = np.asarray(hidden_states, np.float32)
    Wqkv = np.asarray(Wqkv, np.float32)
    Wo = np.asarray(Wo, np.float32)
    crep, srep = _rope_tables()
    tri = (np.arange(128)[None, :] >= np.arange(128)[:, None]).astype(BF)  # [k, q]
    in_maps = []
    for core in range(NCORES):
        b = core // 4
        h0 = (core % 4) * 4
        rq = slice(h0 * 64, (h0 + 4) * 64)
        xT = np.ascontiguousarray(hs[b].T).astype(BF)
        wqkT = np.ascontiguousarray(
            np.concatenate([Wqkv[0:H][rq], Wqkv[H:2 * H][rq]], 0).T
        ).astype(BF)
        wvT = np.ascontiguousarray(Wqkv[2 * H:3 * H][rq].T).astype(BF)
        woT = np.ascontiguousarray(Wo[:, h0 * 64:(h0 + 4) * 64].T).astype(BF)
        in_maps.append(
            dict(xT=xT, wqkT=wqkT, wvT=wvT, woT=woT, crep=crep, srep=srep, trimask=tri)
        )
    return in_maps


def run(hidden_states, Wqkv, Wo, trace=False, trace_cores=None):
    from concourse.bass_utils import run_bass_kernel_spmd

    nc = _get_program()
    in_maps = make_in_maps(hidden_states, Wqkv, Wo)
    res = run_bass_kernel_spmd(
        nc,
        in_maps,
        core_ids=list(range(NCORES)),
        trace=trace,
        trace_cores=trace_cores,
    )
    full = np.zeros((2, S, H), np.float32)
    for core in range(NCORES):
        full[core // 4] += res.results[core]["out"]
    return full, res


def kernel(hidden_states, Wqkv, Wo):
    full, _ = run(hidden_states, Wqkv, Wo)
    return full


# revision 47
# speedup vs baseline: 1.0181x; 1.0181x over previous
# Distributed causal self-attention (RoPE) kernel for one TRN2 chip (8 NeuronCores).
#
# Sharding: data-parallel over batch (B=2) x tensor-parallel over heads
# (16 heads -> 4 per core).  Core i handles batch i//4, heads (i%4)*4 .. +4.
# Each core computes its 4 heads' attention and a partial output projection
# (row-parallel Wo); partials are summed on the host (the only "collective").
#
# Device algorithm (per core, all matmuls bf16 with f32 PSUM accumulation):
#   qkT = wqkT.T @ xT            [512, S]  (q,k in head-transposed layout [d, s])
#   V   = xT.T @ wvT             [S, 256]  (natural layout, +ones column per head)
#   RoPE on qkT (swap via SBUF->SBUF DMA, mults on DVE)
#   per head pair, per 512-wide q quarter, per k-tile (128):
#       scoresT[k, q] = kT_tile.T @ qT      (two heads packed in PE row groups)
#       probsT = exp(scoresT/8)             (ScalarE, no max-subtraction: |s|<=9)
#       triangular 0/1 mask on the diagonal block (DVE)
#       outT[d, q] += Vaug[k-tile].T @ probsT   (65th row accumulates softmax sums)
#   normalize: nao = outT[0:64] * broadcast(1/outT[64])
#   partial_out = naoT.T @ woT   [S, 1024] f32 -> DMA out
import numpy as np
import ml_dtypes

S = 2048
H = 1024
HD = 64
NCORES = 8
BF = ml_dtypes.bfloat16

_CACHE = {}


def _rope_tables():
    inv = 1.0 / (10000.0 ** (np.arange(0, HD, 2, dtype=np.float64) / HD))
    fr = np.outer(np.arange(S, dtype=np.float64), inv)  # [S, 32]
    cosT = np.cos(fr).T.astype(np.float32)  # [32, S]
    sinT = np.sin(fr).T.astype(np.float32)
    crep = np.concatenate([cosT] * 4, 0)  # [128, S]
    srep = np.concatenate([-sinT, sinT, -sinT, sinT], 0)
    return crep.astype(BF), srep.astype(BF)


def _build_program(debug_outputs=False):
    import concourse.bass as bass
    import concourse.mybir as mybir
    import concourse.tile as tile
    from concourse import bacc

    DT = mybir.dt.bfloat16
    F32 = mybir.dt.float32
    EXP = mybir.ActivationFunctionType.Exp

    nc = bacc.Bacc(
        "TRN2",
        target_bir_lowering=False,
        debug=False,
        enable_asserts=False,
        num_devices=NCORES,
    )
    if debug_outputs:
        dbg_qkr_d = nc.declare_dram_parameter("dbg_qkr", [4, 128, S], DT, isOutput=True)
        dbg_v_d = nc.declare_dram_parameter("dbg_v", [128, 16 * 4 * 65], DT, isOutput=True)
        dbg_nao_d = nc.declare_dram_parameter("dbg_nao", [128, 2, S], DT, isOutput=True)
        dbg_rcp_d = nc.declare_dram_parameter("dbg_rcp", [1, 512], F32, isOutput=True)
        dbg_bc_d = nc.declare_dram_parameter("dbg_bc", [64, 512], F32, isOutput=True)
        dbg_ops_d = nc.declare_dram_parameter("dbg_ops", [65, 512], F32, isOutput=True)
        dbg_pr_d = nc.declare_dram_parameter("dbg_pr", [4, 128, 2, 512], DT, isOutput=True)
    xT_d = nc.declare_dram_parameter("xT", [H, S], DT, isOutput=False)
    wqkT_d = nc.declare_dram_parameter("wqkT", [H, 512], DT, isOutput=False)
    wvT_d = nc.declare_dram_parameter("wvT", [H, 256], DT, isOutput=False)
    woT_d = nc.declare_dram_parameter("woT", [256, H], DT, isOutput=False)
    crep_d = nc.declare_dram_parameter("crep", [128, S], DT, isOutput=False)
    srep_d = nc.declare_dram_parameter("srep", [128, S], DT, isOutput=False)
    tri_d = nc.declare_dram_parameter("trimask", [128, 128], DT, isOutput=False)
    out_d = nc.declare_dram_parameter("out", [S, H], F32, isOutput=True)

    with tile.TileContext(nc) as tc:
        with (
            tc.tile_pool(name="consts", bufs=1) as consts,
            tc.tile_pool(name="rtmp", bufs=1) as rtmp,
            tc.tile_pool(name="qkpool", bufs=2) as qkpool,
            tc.tile_pool(name="probs", bufs=6) as prpool,
            tc.tile_pool(name="small", bufs=4) as small,
            tc.tile_pool(name="fout", bufs=3) as foutp,
            tc.tile_pool(name="qkvps", bufs=2, space="PSUM") as qkvps,
            tc.tile_pool(name="spps", bufs=2, space="PSUM") as spps,
            tc.tile_pool(name="opps", bufs=2, space="PSUM") as opps,
        ):
            # ---- load constants / inputs ----
            # weights first (small), then x streamed per contraction tile so
            # the first QKV matmuls start within a few us
            x_ap = xT_d.ap().rearrange("(k c p) s -> k p c s", p=128, c=2)
            wqk_all = consts.tile([128, 8, 512], DT, name="wqk_all")
            nc.sync.dma_start(wqk_all, wqkT_d.ap().rearrange("(c p) o -> p c o", p=128))
            wqk_sb = [wqk_all[:, ct, :] for ct in range(8)]
            x_sb = []
            for k in range(4):
                xt = consts.tile([128, 2, S], DT, name=f"x_sb{k}")
                nc.sync.dma_start(xt, x_ap[k])
                x_sb.append(xt[:, 0, :])
                x_sb.append(xt[:, 1, :])
            wv_sb = consts.tile([128, 8, 256], DT, name="wv_sb")
            nc.sync.dma_start(wv_sb, wvT_d.ap().rearrange("(c p) o -> p c o", p=128))
            wo_sb = consts.tile([128, 2, H], DT, name="wo_sb")
            nc.sync.dma_start(wo_sb, woT_d.ap().rearrange("(c p) o -> p c o", p=128))
            crep_sb = consts.tile([128, S], DT, name="crep_sb")
            nc.sync.dma_start(crep_sb, crep_d.ap())
            srep_sb = consts.tile([128, S], DT, name="srep_sb")
            nc.sync.dma_start(srep_sb, srep_d.ap())
            tri_sb = consts.tile([128, 2, 128], DT, name="tri_sb")
            nc.sync.dma_start(tri_sb[:, 0, :], tri_d.ap())
            nc.sync.dma_start(tri_sb[:, 1, :], tri_d.ap())

            v_sb = consts.tile([128, 16, 4, 65], DT, name="v_sb")
            nc.gpsimd.memset(v_sb[:, :, :, 64:65], 1.0)
            ones_sb = consts.tile([1, 64], DT, name="ones_sb")
            nc.gpsimd.memset(ones_sb, 1.0)
            warm_sb = consts.tile([128, 512], DT, name="warm_sb")
            nc.gpsimd.memset(warm_sb, 0.0)

            def warm_pe(n, rhs=None):
                # scratch matmuls keeping the PE HAM activity window busy while
                # it would otherwise idle (DMA-bound startup, end-of-kernel
                # normalization chain) so real matmuls run at 2.4 GHz instead
                # of the cold 1.2 GHz.  An rhs written by earlier work anchors
                # the scheduler so these land in the intended window.
                wps = qkvps.tile([128, 512], F32, tag="qkvps", name=f"warm{warm_pe.i}")
                warm_pe.i += 1
                if rhs is None:
                    rhs = warm_sb
                for _ in range(n):
                    nc.tensor.matmul(wps, warm_sb[:, 0:128], rhs, start=True, stop=True)
            warm_pe.i = 0

            qk = {}   # raw qkT blocks: 0=q01 1=q23 2=k01 3=k23
            qkw = {}  # half-swapped
            qkr = {}  # roped
            for ot in range(4):
                qk[ot] = consts.tile([128, S], DT, name=f"qk{ot}")
                qkw[ot] = consts.tile([128, S], DT, name=f"qkw{ot}")
                qkr[ot] = consts.tile([128, S], DT, name=f"qkr{ot}")
            # normalized attention output, keyed (ctile, quarter); ctile c
            # holds heads 2c,2c+1 stacked on partitions
            nao = {
                (ct2, c): consts.tile([128, 512], DT, name=f"nao{ct2}{c}")
                for ct2 in range(2)
                for c in range(4)
            }

            def qkv_chunk(ot, scn):
                sl = slice(scn * 512, (scn + 1) * 512)
                ps = qkvps.tile([128, 512], F32, tag="qkvps")
                for ct in range(8):
                    nc.tensor.matmul(
                        ps,
                        wqk_sb[ct][:, ot * 128:(ot + 1) * 128],
                        x_sb[ct][:, sl],
                        start=(ct == 0),
                        stop=(ct == 7),
                    )
                nc.vector.tensor_copy(qk[ot][:, sl], ps)
                # swap head-dim halves (partition permute) via SBUF->SBUF DMA
                for hh in range(2):
                    b0 = hh * 64
                    nc.sync.dma_start(qkw[ot][b0:b0 + 32, sl], qk[ot][b0 + 32:b0 + 64, sl])
                    nc.sync.dma_start(qkw[ot][b0 + 32:b0 + 64, sl], qk[ot][b0:b0 + 32, sl])
                t1 = rtmp.tile([128, 512], DT, tag="rt1", bufs=2)
                t2 = rtmp.tile([128, 512], DT, tag="rt2", bufs=2)
                nc.vector.tensor_mul(t1, qk[ot][:, sl], crep_sb[:, sl])
                nc.vector.tensor_mul(t2, qkw[ot][:, sl], srep_sb[:, sl])
                nc.vector.tensor_add(qkr[ot][:, sl], t1, t2)

            def v_group(st):
                ps = qkvps.tile([128, 256], F32, tag="qkvps")
                for ct in range(8):
                    nc.tensor.matmul(
                        ps,
                        x_sb[ct][:, st * 128:(st + 1) * 128],
                        wv_sb[:, ct, :],
                        start=(ct == 0),
                        stop=(ct == 7),
                    )
                nc.vector.tensor_copy(
                    v_sb[:, st, :, 0:64], ps.rearrange("p (h d) -> p h d", h=4)
                )

            last_pr = [None]

            def attention_quarter(p, c, filler=None):
                # heads hA = 2p, hB = 2p+1 live at partitions 0:64 / 64:128 of
                # qkr[p] (q) and qkr[2+p] (k)
                ops = []
                for hh in range(2):
                    ops.append(
                        opps.tile([65, 512], F32, tag="opps", name=f"op{p}{c}{hh}")
                    )
                nkt = 4 * c + 4
                for kt in range(nkt):
                    if filler is not None and kt in filler:
                        filler[kt]()
                    off = max(0, kt * 128 - c * 512)
                    sp = spps.tile([128, 2, 512], F32, tag="spps")
                    for hh in range(2):
                        nc.tensor.matmul(
                            sp[:, hh, off:512],
                            qkr[2 + p][hh * 64:(hh + 1) * 64, kt * 128:(kt + 1) * 128],
                            qkr[p][hh * 64:(hh + 1) * 64, c * 512 + off:(c + 1) * 512],
                            start=True,
                            stop=True,
                        )
                    pr = prpool.tile([128, 2, 512], DT, tag="probs")
                    last_pr[0] = pr
                    nc.scalar.activation(
                        pr[:, :, off:512], sp[:, :, off:512], EXP, scale=0.125
                    )
                    if kt >= 4 * c:  # diagonal block lives in this quarter
                        dof = (kt - 4 * c) * 128
                        nc.vector.tensor_mul(
                            pr[:, :, dof:dof + 128],
                            pr[:, :, dof:dof + 128],
                            tri_sb,
                        )
                    for hh in range(2):
                        nc.tensor.matmul(
                            ops[hh][:, off:512],
                            v_sb[:, kt, 2 * p + hh, :],
                            pr[:, hh, off:512],
                            start=(kt == 0),
                            stop=(kt == nkt - 1),
                        )
                # single pass over each ops psum tile releases it for the
                # next quarter; the normalization chain then runs off-PSUM
                ao2 = small.tile([64, 2, 512], F32, tag="ao", name=f"ao{p}{c}", bufs=3)
                srow = small.tile([1, 2, 512], F32, tag="srow", bufs=3)
                for hh in range(2):
                    nc.vector.tensor_copy(ao2[:, hh, :], ops[hh][0:64, :])
                    nc.scalar.copy(srow[:, hh, :], ops[hh][64:65, :])
                rcp = small.tile([1, 2, 512], F32, tag="rcp", bufs=3)
                nc.vector.reciprocal_approx_fast(rcp, srow)
                bc = small.tile([64, 2, 512], F32, tag="bc", bufs=3)
                nc.gpsimd.partition_broadcast(bc, rcp)
                for hh in range(2):
                    nc.vector.tensor_mul(
                        nao[(p, c)][hh * 64:(hh + 1) * 64, :],
                        ao2[:, hh, :],
                        bc[:, hh, :],
                    )

            out_ap = out_d.ap()

            def wo_group(c):
                for st in range(4 * c, 4 * c + 4):
                    fo = foutp.tile([128, H], F32, tag="fout")
                    for oh in range(2):
                        fp = qkvps.tile([128, 512], F32, tag="qkvps")
                        for ct2 in range(2):
                            nc.tensor.matmul(
                                fp,
                                nao[(ct2, c)][:, (st % 4) * 128:(st % 4 + 1) * 128],
                                wo_sb[:, ct2, oh * 512:(oh + 1) * 512],
                                start=(ct2 == 0),
                                stop=(ct2 == 1),
                            )
                        if oh == 0:
                            nc.vector.tensor_copy(fo[:, oh * 512:(oh + 1) * 512], fp)
                        else:
                            nc.scalar.copy(fo[:, oh * 512:(oh + 1) * 512], fp)
                    nc.sync.dma_start(out_ap[st * 128:(st + 1) * 128, :], fo)

            # ---- program order (scheduling priority) ----
            # fine-grained interleave: attention quarters start as soon as the
            # q/k columns and V rows they need exist; QKV/V/Wo chunks are the
            # PE filler during the exp-paced attention stream.  The two head
            # pairs alternate quarters so each boundary chain hides inside the
            # other pair's quarter; late wo groups are injected inside the
            # exp-heavy q3 loops.
            warm_pe(22)
            qkv_chunk(0, 0); qkv_chunk(2, 0)
            for st in range(4):
                v_group(st)
            attention_quarter(0, 0)
            qkv_chunk(1, 0); qkv_chunk(3, 0)
            qkv_chunk(0, 1); qkv_chunk(2, 1)
            for st in range(4, 8):
                v_group(st)
            attention_quarter(1, 0)
            qkv_chunk(1, 1); qkv_chunk(3, 1)
            qkv_chunk(0, 2); qkv_chunk(2, 2)
            for st in range(8, 12):
                v_group(st)
            attention_quarter(0, 1)
            qkv_chunk(1, 2); qkv_chunk(3, 2)
            qkv_chunk(0, 3); qkv_chunk(2, 3)
            for st in range(12, 16):
                v_group(st)
            attention_quarter(1, 1)
            qkv_chunk(1, 3); qkv_chunk(3, 3)
            attention_quarter(0, 2)
            wo_group(0)
            attention_quarter(1, 2)
            attention_quarter(0, 3, filler={6: lambda: wo_group(1)})
            attention_quarter(1, 3, filler={6: lambda: wo_group(2)})
            warm_pe(26, rhs=last_pr[0][:, 0, :])
            wo_group(3)

            if debug_outputs:
                for ot in range(4):
                    nc.sync.dma_start(dbg_qkr_d.ap()[ot], qkr[ot])
                nc.sync.dma_start(
                    dbg_v_d.ap(), v_sb.rearrange("p a b c -> p (a b c)")
                )
                for ct2 in range(2):
                    for c in range(4):
                        nc.sync.dma_start(
                            dbg_nao_d.ap()[:, ct2, c * 512:(c + 1) * 512],
                            nao[(ct2, c)],
                        )

    nc.compile()
    return nc


def _get_program(debug_outputs=False):
    key = ("nc", debug_outputs)
    if key not in _CACHE:
        _CACHE[key] = _build_program(debug_outputs)
    return _CACHE[key]


def make_in_maps(hidden_states, Wqkv, Wo):
    hs = np.asarray(hidden_states, np.float32)
    Wqkv = np.asarray(Wqkv, np.float32)
    Wo = np.asarray(Wo, np.float32)
    crep, srep = _rope_tables()
    tri = (np.arange(128)[None, :] >= np.arange(128)[:, None]).astype(BF)  # [k, q]
    in_maps = []
    for core in range(NCORES):
        b = core // 4
        h0 = (core % 4) * 4
        rq = slice(h0 * 64, (h0 + 4) * 64)
        xT = np.ascontiguousarray(hs[b].T).astype(BF)
        wqkT = np.ascontiguousarray(
            np.concatenate([Wqkv[0:H][rq], Wqkv[H:2 * H][rq]], 0).T
        ).astype(BF)
        wvT = np.ascontiguousarray(Wqkv[2 * H:3 * H][rq].T).astype(BF)
        woT = np.ascontiguousarray(Wo[:, h0 * 64:(h0 + 4) * 64].T).astype(BF)
        in_maps.append(
            dict(xT=xT, wqkT=wqkT, wvT=wvT, woT=woT, crep=crep, srep=srep, trimask=tri)
        )
    return in_maps


def run(hidden_states, Wqkv, Wo, trace=False, trace_cores=None):
    from concourse.bass_utils import run_bass_kernel_spmd

    nc = _get_program()
    in_maps = make_in_maps(hidden_states, Wqkv, Wo)
    res = run_bass_kernel_spmd(
        nc,
        in_maps,
        core_ids=list(range(NCORES)),
        trace=trace,
        trace_cores=trace_cores,
    )
    full = np.zeros((2, S, H), np.float32)
    for core in range(NCORES):
        full[core // 4] += res.results[core]["out"]
    return full, res


def kernel(hidden_states, Wqkv, Wo):
    full, _ = run(hidden_states, Wqkv, Wo)
    return full
